# revision 35
# baseline (speedup 1.0000x reference)
"""Causal self-attention (quirky-reshape variant) on 8 TRN2 NeuronCores.

Key structural fact: the reference reshapes (B,S,H*dk) -> (B,H,S,dk) without a
transpose, so head h's Q/K/V come from rows [256h, 256h+256) of the [4096,1024]
projection output (reinterpreted as [4096,64]), and output rows [256h, 256h+256)
depend only on head h.  With 2 heads per core the problem is embarrassingly
parallel: core i consumes x rows [512i, 512i+512) + full weights and produces
output rows [512i, 512i+512).  No collectives.

v2 schedule (vs. v1 baseline):
  - Phase order: Q proj -> Q rope (DVE) || K proj (PE) -> K rope (chunked)
    -> first scores start ~31us; V proj + DRAM-roundtrip reshape overlap the
    early attention stream (deep pch buffering rides over the V wait).
  - Causal masking moved off PSUM: exp full chunks on ScalarE, then multiply
    the diagonal sub-blocks of the exp'd P by a 0/1 mask on DVE in SBUF
    (cheaper than f32 adds on PSUM, and unclogs the PSUM port for ACT).
  - Softmax normalize: reciprocal_approx_fast from PSUM -> gpsimd
    partition_broadcast (idle engine) -> single DVE multiply; double-buffered.
  - All projection PSUM->SBUF copies on ScalarE (idle during lead-in);
    ct/psy copies on DVE (4x bf16 mode).
  - Output projection emitted per (head, row-half) as soon as its 4 q-blocks
    are normalized, overlapping the attention tail; y DMA'd per 128-row group.
"""

import os

os.environ.setdefault("JAX_PLATFORMS", "cpu")

import numpy as np
import ml_dtypes

D = 1024          # d_model
H = 16            # heads
DK = 64           # head dim
S = 4096          # seq len
NC_N = 8          # cores
RPC = 512         # x rows per core
HPC = 2           # heads per core
NT_SK = 32        # sk tiles of 128 per head
ROPE_THETA = 10000.0

_CACHE = {}

# every Nth score chunk computes exp on DVE via (1 + u + u²/2)^64 (u = s/512)
# instead of ScalarE's LUT exp; 0 disables.  ScalarE exp is the attention
# phase's rate limiter, DVE has slack.
DVE_EXP_EVERY = int(os.environ.get("DVE_EXP_EVERY", "0"))


def _register_dve_exp():
    """Register the 2-instruction DVE exp chain (documented custom-DVE
    extension point: per-NEFF uop table, no firmware change)."""
    from concourse import dve_ops
    from concourse.dve_spec import Spec, Src0, One, sq, C0, C1, lower
    from concourse.dve_spec import _has_src1 as has_src1
    from concourse.dve_uop import DveOpSpec

    have = {op.name: op for op in dve_ops.OPS}
    if "ATTN_EXP_BASE" in have:
        return have["ATTN_EXP_BASE"], have["ATTN_EXP_SQ6"]

    u = Src0 * C0
    base = Spec(
        body=(One + u) + sq(u) * C1,
        reference=lambda in0, in1, c0, c1, c2:
            (1.0 + in0 * c0) + (in0 * c0) ** 2 * c1,
    )
    sq6 = Spec(
        body=sq(sq(sq(sq(sq(sq(Src0)))))),
        reference=lambda in0, in1, c0, c1, c2: in0 ** 64,
    )
    out = []
    for name, sp in (("ATTN_EXP_BASE", base), ("ATTN_EXP_SQ6", sq6)):
        shas = {}
        for ver in ("v3", "v4"):
            try:
                s = DveOpSpec(name=name, opcode=1, uops=lower(sp, ver=ver),
                              rd1_en=has_src1(sp))
                shas[ver] = s.sha(ver)
            except Exception:
                pass
        op = dve_ops.DveOp(name, sp, subdim=False, uops_sha=shas)
        dve_ops.OPS.append(op)
        dve_ops.CUSTOM_DVE_SPECS[name] = sp
        dve_ops._SUB_OPCODE_FOR_NAME[name] = (
            max(dve_ops._SUB_OPCODE_FOR_NAME.values()) + 1)
        out.append(op)
    return out[0], out[1]


def _deint_perm():
    """o' -> o source index: within each 64-block, evens first then odds."""
    d_order = list(range(0, DK, 2)) + list(range(1, DK, 2))  # position d' -> d
    perm = np.zeros(D, dtype=np.int64)
    for c in range(H):
        for dp, d in enumerate(d_order):
            perm[c * DK + dp] = c * DK + d
    return perm


def _sbuf_shuffle(wT):
    """[1024, 1024] -> [128, 8*1024] so each SBUF partition line is one
    contiguous 16KB DRAM run (128 fat DMA descriptors instead of 1024)."""
    return np.ascontiguousarray(
        wT.reshape(8, 128, wT.shape[1]).transpose(1, 0, 2).reshape(128, -1))


def _host_arrays(Wq, Wk, Wv, Wo):
    bf = ml_dtypes.bfloat16
    perm = _deint_perm()
    wqT = _sbuf_shuffle(np.ascontiguousarray(Wq[perm, :].T).astype(bf))
    wkT = _sbuf_shuffle(np.ascontiguousarray(Wk[perm, :].T).astype(bf))
    wvT = _sbuf_shuffle(np.ascontiguousarray(Wv.T).astype(bf))
    woT = _sbuf_shuffle(np.ascontiguousarray(Wo.T).astype(bf))

    # RoPE tables in the [Aev, Aod, Bev, Bod] partition grouping (32 rows each;
    # identical for both heads since the angle depends only on s).
    # Both Q and K are stored c-major (u = 256*c + r <-> s = 16*r + c), so one
    # u-ordered table pair serves both.
    j = np.arange(0, DK, 2, dtype=np.float64) / DK
    inv_freq = 1.0 / (ROPE_THETA ** j)                     # [32]
    u = np.arange(S)
    s_of_u = 16 * (u % 256) + u // 256
    angq = np.outer(inv_freq, s_of_u)                      # [32, S] u-ordered
    angk = np.outer(inv_freq, np.arange(S))                # [32, S] s-ordered
    csq1 = np.concatenate([np.cos(angq)] * 4, 0).astype(bf)
    csq2 = np.concatenate([-np.sin(angq), np.sin(angq)] * 2, 0).astype(bf)
    csk1 = np.concatenate([np.cos(angk)] * 4, 0).astype(bf)
    csk2 = np.concatenate([-np.sin(angk), np.sin(angk)] * 2, 0).astype(bf)
    # 0/1 triangle mask applied multiplicatively to exp'd P:
    # row p = local sk (plain), col j = 8*cq + rq (c-major sq)
    p = np.arange(128)
    cq, rq = np.arange(128) // 8, np.arange(128) % 8
    sq_loc = 16 * rq + cq                                  # [128]
    tri01 = np.where(p[:, None] <= sq_loc[None, :], 1.0, 0.0).astype(bf)
    # duplicated for both chunk slots so one gpsimd op masks the whole chunk
    tri01 = np.concatenate([tri01, tri01], axis=1)
    return wqT, wkT, wvT, woT, csq1, csq2, csk1, csk2, tri01


def _build_program(dbg=False):
    import concourse.bass as bass
    import concourse.tile as tile
    from concourse import bacc, mybir

    EXPB = EXPS = None
    if DVE_EXP_EVERY:
        EXPB, EXPS = _register_dve_exp()

    f32 = mybir.dt.float32
    bf16 = mybir.dt.bfloat16
    EXP = mybir.ActivationFunctionType.Exp
    CPY = mybir.ActivationFunctionType.Copy

    nc = bacc.Bacc("TRN2", target_bir_lowering=False, debug=False,
                   num_devices=NC_N)

    xT = nc.dram_tensor("xT", [128, 8 * RPC], bf16, kind="ExternalInput").ap()
    wq = nc.dram_tensor("wqT", [128, 8 * D], bf16, kind="ExternalInput").ap()
    wk = nc.dram_tensor("wkT", [128, 8 * D], bf16, kind="ExternalInput").ap()
    wv = nc.dram_tensor("wvT", [128, 8 * D], bf16, kind="ExternalInput").ap()
    wo = nc.dram_tensor("woT", [128, 8 * D], bf16, kind="ExternalInput").ap()
    cs1d = nc.dram_tensor("cs1", [128, S], bf16, kind="ExternalInput").ap()
    cs2d = nc.dram_tensor("cs2", [128, S], bf16, kind="ExternalInput").ap()
    cs3d = nc.dram_tensor("cs3", [128, S], bf16, kind="ExternalInput").ap()
    cs4d = nc.dram_tensor("cs4", [128, S], bf16, kind="ExternalInput").ap()
    trid = nc.dram_tensor("tri", [128, 256], bf16, kind="ExternalInput").ap()
    y = nc.dram_tensor("y", [RPC, D], f32, kind="ExternalOutput").ap()
    vfd = nc.dram_tensor("vflat_scratch", [RPC, D], bf16).ap()

    with tile.TileContext(nc) as tc:
        with (
            tc.tile_pool(name="big", bufs=3) as big,        # wq/wk/wv -> outTA/outTB/y_sb
            tc.tile_pool(name="wo", bufs=1) as wop,
            tc.tile_pool(name="xp", bufs=1) as xp,
            tc.tile_pool(name="qk", bufs=2) as qkp,          # qraw, kraw (become rot in place)
            tc.tile_pool(name="cs", bufs=4) as csp,          # rope tables
            tc.tile_pool(name="sw", bufs=2) as swp,          # K-rope swap scratch
            tc.tile_pool(name="swq", bufs=1) as swqp,        # Q-rope swap scratch
            tc.tile_pool(name="vf", bufs=1) as vfp,
            tc.tile_pool(name="vsb", bufs=2) as vsbp,
            tc.tile_pool(name="stg", bufs=1) as stgp,        # K r-tail staging
            tc.tile_pool(name="mask", bufs=1) as maskp,
            tc.tile_pool(name="pp", bufs=18) as ppool,       # exp'd P chunks
            tc.tile_pool(name="ct", bufs=4) as ctp,          # concatT per (h, rt)
            tc.tile_pool(name="norm", bufs=2) as normp,      # dn row + bcast block
        ):
            # ---------- phase 0: loads ----------
            # DRAM pre-shuffled on host so every 512-col chunk is a 1KB
            # contiguous run per partition line.  All loads go on the sync
            # ring (scalar-ring dma_starts backpressure the ScalarE FIFO
            # and stall the PSUM-evac copies behind them).  Emission order
            # is first-use order for the K-first phase schedule:
            # x, wk, wq, cs34, wv, cs12, tri (wo rides behind the gather).
            def loadc(dst, src, n_chunks):
                w = dst.shape[-1] // n_chunks
                for ch in range(n_chunks):
                    nc.sync.dma_start(dst[:, ch * w:(ch + 1) * w],
                                      src[:, ch * w:(ch + 1) * w])

            xsb = xp.tile([128, 8 * RPC], bf16, tag="x")           # [p, kt*512+r]
            wq_sb = big.tile([128, 8 * D], bf16, tag="big", name="w_wq")
            wk_sb = big.tile([128, 8 * D], bf16, tag="big", name="w_wk")
            wv_sb = big.tile([128, 8 * D], bf16, tag="big", name="w_wv")
            wo_sb = wop.tile([128, 8 * D], bf16, tag="wo")
            # interleave x and wk chunks so K proj's kt=0 inputs land first
            for kt in range(8):
                nc.sync.dma_start(xsb[:, kt * RPC:(kt + 1) * RPC],
                                  xT[:, kt * RPC:(kt + 1) * RPC])
                nc.sync.dma_start(wk_sb[:, kt * D: kt * D + 512],
                                  wk[:, kt * D: kt * D + 512])
                nc.sync.dma_start(wk_sb[:, kt * D + 512:(kt + 1) * D],
                                  wk[:, kt * D + 512:(kt + 1) * D])
            loadc(wq_sb[:], wq, 16)
            cs3_sb = csp.tile([128, S], bf16, tag="cs")
            loadc(cs3_sb[:], cs3d, 8)
            cs4_sb = csp.tile([128, S], bf16, tag="cs")
            loadc(cs4_sb[:], cs4d, 8)
            loadc(wv_sb[:], wv, 16)
            cs1_sb = csp.tile([128, S], bf16, tag="cs")
            loadc(cs1_sb[:], cs1d, 8)
            cs2_sb = csp.tile([128, S], bf16, tag="cs")
            loadc(cs2_sb[:], cs2d, 8)
            tri_sb = maskp.tile([128, 256], bf16, tag="mask")
            nc.sync.dma_start(tri_sb[:], trid[:])

            qraw = qkp.tile([128, S], bf16, tag="qk")   # [Aev,Aod,Bev,Bod] x s
            kraw = qkp.tile([128, S], bf16, tag="qk")

            # vsb tiles early (gpsimd memset of the ones column is off-path)
            vsbs = []
            for h in range(HPC):
                vsb = vsbp.tile([128, 65 * NT_SK], bf16, tag="vsb")
                nc.gpsimd.memset(vsb[:], 1.0)   # ones col at 65T+64 survives
                vsbs.append(vsb)

            from contextlib import ExitStack
            _ps_stack = ExitStack()
            with _ps_stack:
                # projections get all 8 PSUM banks; the scores pool only
                # opens once the projection pool closes (pre-attention)
                with tc.tile_pool(name="psproj", bufs=8, space="PSUM") as psp:
                    # ---------- phase 1b: K projection FIRST (s-ordered
                    # storage; strided copies split across both engines).
                    # K leads so rope chunk 0 + the first QK scores come as
                    # early as possible; Q/V projections then overlap the
                    # rope chain on DVE. ----------
                    kv_raw = kraw[:].rearrange("p (r c) -> p r c", c=16)
                    stgk = stgp.tile([128, 1024], bf16, tag="stg")

                    def kcopy2(pq, ot):
                        # pass 2: r in [64, 192) — emitted 4 ots behind so
                        # rope chunk 0 (which only needs r < 64) can jump the
                        # copy queues
                        c0 = 2 * ot
                        nc.scalar.activation(kv_raw[0:64, 64:192, c0],
                                             pq[0:64, 64:192], CPY)
                        nc.vector.tensor_copy(kv_raw[64:128, 64:192, c0],
                                              pq[0:64, 320:448])
                        nc.vector.tensor_copy(kv_raw[0:64, 64:192, c0 + 1],
                                              pq[64:128, 64:192])
                        nc.scalar.activation(kv_raw[64:128, 64:192, c0 + 1],
                                             pq[64:128, 320:448], CPY)

                    kpqs = []
                    for ot in range(8):
                        if ot >= 4:
                            kcopy2(kpqs[ot - 4], ot - 4)
                        pq = psp.tile([128, RPC], f32, tag="ps")
                        for kt in range(8):
                            nc.tensor.matmul(
                                pq[:],
                                wk_sb[:, kt * D + ot * 128: kt * D + ot * 128 + 128],
                                xsb[:, kt * RPC: (kt + 1) * RPC],
                                start=(kt == 0), stop=(kt == 7),
                            )
                        c0 = 2 * ot
                        # pass 1: r < 64 (s < 1024: rope chunk 0's input),
                        # split across both engines
                        nc.scalar.activation(kv_raw[0:64, 0:64, c0],
                                             pq[0:64, 0:64], CPY)
                        nc.vector.tensor_copy(kv_raw[64:128, 0:64, c0],
                                              pq[0:64, 256:320])
                        nc.vector.tensor_copy(kv_raw[0:64, 0:64, c0 + 1],
                                              pq[64:128, 0:64])
                        nc.scalar.activation(kv_raw[64:128, 0:64, c0 + 1],
                                             pq[64:128, 256:320], CPY)
                        # r-tail [192, 256) staged contiguously, scattered
                        # later under the attention stream
                        stgs = stgk[:, 128 * ot: 128 * ot + 128].rearrange(
                            "p (g r) -> p g r", g=2)
                        pqs = pq[:].rearrange("p (g r) -> p g r", g=2)
                        nc.scalar.activation(stgs[:], pqs[:, :, 192:256], CPY)
                        kpqs.append(pq)
                    for ot in range(4, 8):
                        kcopy2(kpqs[ot], ot)
                    # K rope chunks 0-2 (s < 3072, covers q-blocks 0-5);
                    # chunk 3 is deferred with the r-tail scatter
                    kv_stg = stgk[:].rearrange("p (ot g r) -> p ot g r",
                                               ot=8, g=2)

                    def emit_krope(kc):
                        lo, hi = 1024 * kc, 1024 * (kc + 1)
                        swk = swp.tile([128, 1024], bf16, tag="sw",
                                       name=f"swk{kc}")
                        nc.vector.tensor_copy(swk[0:32, :], kraw[32:64, lo:hi])
                        nc.vector.tensor_copy(swk[32:64, :], kraw[0:32, lo:hi])
                        nc.vector.tensor_copy(swk[64:96, :], kraw[96:128, lo:hi])
                        nc.vector.tensor_copy(swk[96:128, :], kraw[64:96, lo:hi])
                        nc.vector.tensor_mul(swk[:], swk[:], cs4_sb[:, lo:hi])
                        nc.vector.tensor_mul(kraw[:, lo:hi], kraw[:, lo:hi],
                                             cs3_sb[:, lo:hi])
                        nc.vector.tensor_add(kraw[:, lo:hi], kraw[:, lo:hi],
                                             swk[:])

                    emit_krope(0)

                    # ---------- phase 1a: Q projection (c-major storage,
                    # contiguous copies, split 2-ScalarE / 2-DVE) ----------
                    for ot in range(8):
                        pq = psp.tile([128, RPC], f32, tag="ps")
                        for kt in range(8):
                            nc.tensor.matmul(
                                pq[:],
                                wq_sb[:, kt * D + ot * 128: kt * D + ot * 128 + 128],
                                xsb[:, kt * RPC: (kt + 1) * RPC],
                                start=(kt == 0), stop=(kt == 7),
                            )
                        c0 = 2 * ot
                        u0, u1 = 256 * c0, 256 * (c0 + 1)
                        nc.scalar.activation(qraw[0:64, u0:u0 + 256],
                                             pq[0:64, 0:256], CPY)
                        nc.vector.tensor_copy(qraw[64:128, u0:u0 + 256],
                                              pq[0:64, 256:512])
                        nc.vector.tensor_copy(qraw[0:64, u1:u1 + 256],
                                              pq[64:128, 0:256])
                        nc.scalar.activation(qraw[64:128, u1:u1 + 256],
                                             pq[64:128, 256:512], CPY)
                    # Q rope, full width, on the first-exp critical path:
                    # split the serial chain across DVE and the idle GpSimd
                    # (2+2 swap copies in parallel, then the two multiplies
                    # on different engines) to cut ~3.5us off the lead-in
                    swq = swqp.tile([128, S], bf16, tag="swq", name="swq")
                    nc.vector.tensor_copy(swq[0:32, :], qraw[32:64, :])
                    nc.gpsimd.tensor_copy(swq[32:64, :], qraw[0:32, :])
                    nc.vector.tensor_copy(swq[64:96, :], qraw[96:128, :])
                    nc.gpsimd.tensor_copy(swq[96:128, :], qraw[64:96, :])
                    nc.vector.tensor_mul(swq[:], swq[:], cs2_sb[:])
                    nc.gpsimd.tensor_mul(qraw[:], qraw[:], cs1_sb[:])
                    nc.vector.tensor_add(qraw[:], qraw[:], swq[:])

                    # (V projection moved into the attention prelude below,
                    # interleaved with the first chunks' QK+exp)

                    def scat(ot):
                        c0 = 2 * ot
                        nc.vector.tensor_copy(kv_raw[0:64, 192:256, c0],
                                              kv_stg[0:64, ot, 0, :])
                        nc.vector.tensor_copy(kv_raw[64:128, 192:256, c0],
                                              kv_stg[0:64, ot, 1, :])
                        nc.vector.tensor_copy(kv_raw[0:64, 192:256, c0 + 1],
                                              kv_stg[64:128, ot, 0, :])
                        nc.vector.tensor_copy(kv_raw[64:128, 192:256, c0 + 1],
                                              kv_stg[64:128, ot, 1, :])

                    qrot, krot = qraw, kraw

                # ---------- phase 4: attention ----------
                pssc = _ps_stack.enter_context(
                    tc.tile_pool(name="pssc", bufs=2, space="PSUM"))
                qv = [qrot[64 * h: 64 * h + 64, :].rearrange(
                    "p (c r) -> p c r", c=16) for h in range(HPC)]
                triv = tri_sb[:].rearrange("p (j c r) -> p j c r",
                                           j=2, r=8)

                all_chunks = []
                for b in range(8):
                    nt = 4 * (b + 1)
                    slots = [(t, h) for t in range(nt) for h in range(HPC)]
                    for i in range(0, len(slots), 2):
                        all_chunks.append((b, slots[i:i + 2],
                                           i + 2 >= len(slots)))

                # deadline-ordered deferred DVE work: K-rope chunks ahead of
                # the sk tiles that need them, K r-tail scatter in between
                dve_bg = ([lambda: emit_krope(1), lambda: emit_krope(2)]
                          + [(lambda ot=ot: scat(ot)) for ot in range(8)]
                          + [lambda: emit_krope(3)])

                def emit_chunk(ci, b, chunk):
                    ps = pssc.tile([128, 1024], f32, tag="sc")
                    pch = ppool.tile([128, 1024], bf16, tag="pp")
                    # slot layout: uniform c-major-32 groups; valid r-range
                    # [rmin, 32) per c-group, garbage at [0, rmin)
                    for j, (t, h) in enumerate(chunk):
                        rmin = 8 * max(0, t - 4 * b)
                        psv = ps[:, 512 * j: 512 * (j + 1)].rearrange(
                            "p (c r) -> p c r", r=32)
                        nc.tensor.matmul(
                            psv[:, :, rmin:32],
                            krot[64 * h: 64 * h + 64,
                                 128 * t: 128 * t + 128],
                            qv[h][:, :, 32 * b + rmin: 32 * (b + 1)],
                            start=True, stop=True,
                        )
                    rmin0 = 8 * max(0, chunk[0][0] - 4 * b)
                    if rmin0:
                        # diagonal chunks: skip exp'ing the [0, rmin)
                        # garbage cols (never read downstream)
                        psv4 = ps[:].rearrange("p (j c r) -> p j c r",
                                               j=2, r=32)
                        pcv4 = pch[:].rearrange("p (j c r) -> p j c r",
                                                j=2, r=32)
                        nc.scalar.activation(pcv4[:, :, :, rmin0:32],
                                             psv4[:, :, :, rmin0:32],
                                             EXP, scale=0.125)
                    else:
                        nc.scalar.activation(pch[:], ps[:], EXP,
                                             scale=0.125)
                    if chunk[0][0] >= 4 * b:
                        # both slots share t -> one fused mask multiply
                        pm = pch[:].rearrange(
                            "p (j c r) -> p j c r", j=2,
                            r=32)[:, :, :, rmin0:rmin0 + 8]
                        nc.gpsimd.tensor_mul(pm, pm, triv)
                    if dve_bg and ci % 2 == 0:
                        dve_bg.pop(0)()
                    return pch

                # ---------- phase 1v (interleaved): V projection + reshape
                # woven between the first N_EARLY chunks' QK+exp so the
                # ScalarE exp stream starts ~25us earlier; those chunks'
                # PV waits in pvq (pch pool keeps them live) until the vsb
                # tiles land.  rt order (0,2,1,3): both heads' first halves
                # reach vsb first.  Reshape goes through DRAM (gather).
                N_EARLY = 12
                pvq = []
                ecur = [0]

                def emit_early(n):
                    while n > 0 and ecur[0] < N_EARLY:
                        ci = ecur[0]
                        eb, echunk, elast = all_chunks[ci]
                        pch = emit_chunk(ci, eb, echunk)
                        pvq.append((eb, echunk, pch, elast))
                        ecur[0] += 1
                        n -= 1

                vflat = vfp.tile([128, 4 * D], bf16, tag="vf")
                vfd_v = vfd.rearrange("(rt p) o -> p rt o", p=128)
                vld = vfd.rearrange("(h T a) (c d) -> h a c T d", h=2,
                                    T=NT_SK, a=8, c=16, d=DK)
                _vps = ExitStack()
                vpsp = _vps.enter_context(
                    tc.tile_pool(name="psv", bufs=2, space="PSUM"))
                for rt in (0, 2, 1, 3):
                    for ob in range(2):
                        pv = vpsp.tile([128, 512], f32, tag="psv")
                        for kt in range(8):
                            nc.tensor.matmul(
                                pv[:],
                                xsb[:, kt * RPC + rt * 128: kt * RPC + rt * 128 + 128],
                                wv_sb[:, kt * D + ob * 512: kt * D + ob * 512 + 512],
                                start=(kt == 0), stop=(kt == 7),
                            )
                        # PSUM evac on DVE: ScalarE now runs the exp stream
                        nc.vector.tensor_copy(
                            vflat[:, rt * D + ob * 512: rt * D + ob * 512 + 512],
                            pv[:])
                        emit_early(1)
                    nc.sync.dma_start(vfd_v[:, rt, :],
                                      vflat[:].rearrange(
                                          "p (rt o) -> p rt o", rt=4)[:, rt, :])
                    # quirky map: vfd rows [256h+128*half, +128) = head h,
                    # tiles T in [16*half, 16*half+16)
                    h, half = divmod(rt, 2)
                    T0 = 16 * half
                    dstv = vsbs[h][:].rearrange("(a c) (T d) -> a c T d",
                                                a=8, c=16, T=NT_SK, d=65)
                    for a in range(8):
                        nc.sync.dma_start(dstv[a, :, T0:T0 + 16, 0:DK],
                                          vld[h, a, :, T0:T0 + 16, :])
                    emit_early(1)
                emit_early(N_EARLY)
                _vps.close()
                # wo rides behind the V gather: first needed at the b=4
                # oproj drip, well after the gather drains
                loadc(wo_sb[:], wo, 16)

                with tc.tile_pool(name="psout", bufs=4, space="PSUM") as psout:
                    y_sb = big.tile([128, 4 * D], f32, tag="big")
                    yv = y.rearrange("(g p) o -> p g o", p=128)

                    # normalize writes land directly in concatT (ct) layout:
                    # ct[(c%2)*64+d, 128*(c//2) + r%128] per (h, rt=r//128),
                    # so the o-projection needs no gather copies at all
                    cts = {}
                    for h in range(HPC):
                        for rt in range(2):
                            cts[(h, rt)] = ctp.tile([128, 8 * 128], bf16,
                                                    tag="ct",
                                                    name=f"ct{h}_{rt}")

                    outps = {}
                    opvs = {}

                    def ensure_outp(b):
                        if b in outps:
                            return
                        outps[b] = [psout.tile([65, 512], f32, tag="out",
                                               name=f"outp{b}_{hh}")
                                    for hh in range(HPC)]
                        opvs[b] = [outps[b][hh][:].rearrange(
                            "p (c r) -> p c r", r=32) for hh in range(HPC)]

                    def emit_pv(b, chunk, pch):
                        nt = 4 * (b + 1)
                        for j, (t, h) in enumerate(chunk):
                            rmin = 8 * max(0, t - 4 * b)
                            pcv = pch[:, 512 * j: 512 * (j + 1)].rearrange(
                                "p (c r) -> p c r", r=32)
                            nc.tensor.matmul(
                                opvs[b][h][:, :, rmin:32],
                                vsbs[h][:, 65 * t: 65 * t + 65],
                                pcv[:, :, rmin:32],
                                start=(t == 0), stop=(t == nt - 1),
                            )

                    def emit_norm(b, h):
                        # row 64 of outp holds the softmax denominators (ones
                        # column of the [V|1] stationary); broadcast 1/d on
                        # the idle GpSimd and scale straight into ct layout
                        rt, ro = b // 4, 32 * (b % 4)
                        outp = outps[b][h]
                        nrm = normp.tile([128, 1024], f32, tag="norm",
                                         name=f"nrm{b}_{h}")
                        nc.vector.tensor_copy(nrm[0:1, 512:1024],
                                              outp[64:65, :])
                        nc.vector.reciprocal_approx_fast(
                            out=nrm[0:1, 0:512], in_=nrm[0:1, 512:1024])
                        nc.vector.stream_shuffle(nrm[64:96, 0:512],
                                                 nrm[0:32, 0:512], [0] * 32)
                        nc.vector.stream_shuffle(nrm[96:128, 0:512],
                                                 nrm[0:32, 0:512], [0] * 32)
                        # outp col = 32c + r with c = 2*tp + c2; ct col =
                        # 128*tp + 32*(b%4) + r, partition half by c2
                        ctv = cts[(h, rt)][:].rearrange(
                            "p (tp j) -> p tp j", j=128)
                        opv2 = outp[:].rearrange("p (tp c2 r) -> p tp c2 r",
                                                 c2=2, r=32)
                        bcv = nrm[64:128, 0:512].rearrange(
                            "p (tp c2 r) -> p tp c2 r", c2=2, r=32)
                        for c2 in range(2):
                            nc.vector.tensor_mul(
                                ctv[64 * c2: 64 * c2 + 64, :, ro:ro + 32],
                                opv2[0:64, :, c2], bcv[:, :, c2])

                    oproj_bg = []   # deferred o-proj work, drip-fed to PE

                    def make_oproj(h, rt):
                        g = 2 * h + rt
                        ct = cts[(h, rt)]
                        work = []
                        cell = {}   # py allocated lazily at first matmul so a
                        # PSUM slot is only held while the chain is in flight
                        for ob in range(2):
                            for tp in range(8):
                                def mm(tp=tp, ob=ob):
                                    if tp == 0:
                                        cell[ob] = psout.tile(
                                            [128, 512], f32, tag="out",
                                            name=f"py{g}_{ob}")
                                    nc.tensor.matmul(
                                        cell[ob][:],
                                        ct[:, 128 * tp: 128 * tp + 128],
                                        wo_sb[:, tp * D + ob * 512:
                                              tp * D + ob * 512 + 512],
                                        start=(tp == 0), stop=(tp == 7),
                                    )
                                work.append(mm)

                            def fin(ob=ob):
                                nc.vector.tensor_copy(
                                    y_sb[:, g * D + ob * 512:
                                         g * D + ob * 512 + 512],
                                    cell[ob][:])
                                if ob == 1:
                                    nc.sync.dma_start(
                                        yv[:, g, :], y_sb[:, g * D:(g + 1) * D])
                            work.append(fin)
                        return work

                    def flush_pv(n):
                        # drain the oldest queued chunks' PV (in order);
                        # norm fires on each block's last chunk
                        while pvq and n > 0:
                            pb, pchunk, ppch, plast = pvq.pop(0)
                            ensure_outp(pb)
                            emit_pv(pb, pchunk, ppch)
                            if plast and pb != 7:
                                for hh in range(HPC):
                                    emit_norm(pb, hh)
                                if pb == 3:
                                    oproj_bg.extend(make_oproj(0, 0))
                                    oproj_bg.extend(make_oproj(1, 0))
                            n -= 1

                    for ci in range(N_EARLY, len(all_chunks)):
                        b, chunk, is_last = all_chunks[ci]
                        pch = emit_chunk(ci, b, chunk)
                        pvq.append((b, chunk, pch, is_last))
                        # steady state keeps lag-1; the early backlog drains
                        # at 2 per chunk
                        flush_pv(2 if len(pvq) > 6 else
                                 (1 if len(pvq) > 1 else 0))
                        if oproj_bg and b >= 4 and ci % 2 == 0:
                            oproj_bg.pop(0)()
                    flush_pv(len(pvq))
                    while oproj_bg:
                        oproj_bg.pop(0)()
                    # tail: both norms first, then interleave the two final
                    # oproj groups so each group's LDW/sem waits hide under
                    # the other group's matmuls
                    for hh in range(HPC):
                        emit_norm(7, hh)
                    w0 = make_oproj(0, 1)
                    w1 = make_oproj(1, 1)
                    for wa, wb in zip(w0, w1):
                        wa()
                        wb()

    nc.compile()
    return nc


def kernel(**inputs):
    x = np.asarray(inputs["x"], dtype=np.float32)     # [1, 4096, 1024]
    Wq = np.asarray(inputs["Wq"], dtype=np.float32)
    Wk = np.asarray(inputs["Wk"], dtype=np.float32)
    Wv = np.asarray(inputs["Wv"], dtype=np.float32)
    Wo = np.asarray(inputs["Wo"], dtype=np.float32)
    # biases are structurally zero in this problem; fold anyway if nonzero
    for bn in ("bq", "bk", "bv", "bo"):
        bv_ = np.asarray(inputs.get(bn, 0.0))
        assert np.all(bv_ == 0.0), f"{bn} nonzero: unsupported"

    from concourse.bass_utils import run_bass_kernel_spmd

    if "nc" not in _CACHE:
        _CACHE["nc"] = _build_program()
    nc = _CACHE["nc"]

    bf = ml_dtypes.bfloat16
    wqT, wkT, wvT, woT, csq1, csq2, csk1, csk2, tri01 = _host_arrays(
        Wq, Wk, Wv, Wo)
    shared = {"wqT": wqT, "wkT": wkT, "wvT": wvT, "woT": woT,
              "cs1": csq1, "cs2": csq2, "cs3": csk1, "cs4": csk2,
              "tri": tri01}
    xf = x.reshape(S, D)
    in_maps = []
    for i in range(NC_N):
        xTi = _sbuf_shuffle(
            np.ascontiguousarray(xf[i * RPC:(i + 1) * RPC, :].T).astype(bf))
        in_maps.append(dict(shared, xT=xTi))

    trace = bool(int(os.environ.get("BASS_KERNEL_TRACE", "0")))
    res = run_bass_kernel_spmd(nc, in_maps, core_ids=list(range(NC_N)),
                               trace=trace)
    _CACHE["last_res"] = res
    if trace and res.exec_time_ns is not None:
        print(f"HW exec time: {res.exec_time_ns} ns")
        _CACHE["exec_time_ns"] = res.exec_time_ns
        _CACHE["trace"] = res.instructions_and_trace
    out = np.concatenate([res.results[i]["y"] for i in range(NC_N)], axis=0)
    return out.reshape(1, S, D).astype(np.float32)



# revision 36
# speedup vs baseline: 1.1301x; 1.1301x over previous
"""Causal self-attention (quirky-reshape variant) on 8 TRN2 NeuronCores.

Key structural fact: the reference reshapes (B,S,H*dk) -> (B,H,S,dk) without a
transpose, so head h's Q/K/V come from rows [256h, 256h+256) of the [4096,1024]
projection output (reinterpreted as [4096,64]), and output rows [256h, 256h+256)
depend only on head h.  With 2 heads per core the problem is embarrassingly
parallel: core i consumes x rows [512i, 512i+512) + full weights and produces
output rows [512i, 512i+512).  No collectives.

v2 schedule (vs. v1 baseline):
  - Phase order: Q proj -> Q rope (DVE) || K proj (PE) -> K rope (chunked)
    -> first scores start ~31us; V proj + DRAM-roundtrip reshape overlap the
    early attention stream (deep pch buffering rides over the V wait).
  - Causal masking moved off PSUM: exp full chunks on ScalarE, then multiply
    the diagonal sub-blocks of the exp'd P by a 0/1 mask on DVE in SBUF
    (cheaper than f32 adds on PSUM, and unclogs the PSUM port for ACT).
  - Softmax normalize: reciprocal_approx_fast from PSUM -> gpsimd
    partition_broadcast (idle engine) -> single DVE multiply; double-buffered.
  - All projection PSUM->SBUF copies on ScalarE (idle during lead-in);
    ct/psy copies on DVE (4x bf16 mode).
  - Output projection emitted per (head, row-half) as soon as its 4 q-blocks
    are normalized, overlapping the attention tail; y DMA'd per 128-row group.
"""

import os

os.environ.setdefault("JAX_PLATFORMS", "cpu")

import numpy as np
import ml_dtypes

D = 1024          # d_model
H = 16            # heads
DK = 64           # head dim
S = 4096          # seq len
NC_N = 8          # cores
RPC = 512         # x rows per core
HPC = 2           # heads per core
NT_SK = 32        # sk tiles of 128 per head
ROPE_THETA = 10000.0

_CACHE = {}

# every Nth score chunk computes exp on DVE via (1 + u + u²/2)^64 (u = s/512)
# instead of ScalarE's LUT exp; 0 disables.  ScalarE exp is the attention
# phase's rate limiter, DVE has slack.
DVE_EXP_EVERY = int(os.environ.get("DVE_EXP_EVERY", "0"))


def _register_dve_exp():
    """Register the 2-instruction DVE exp chain (documented custom-DVE
    extension point: per-NEFF uop table, no firmware change)."""
    from concourse import dve_ops
    from concourse.dve_spec import Spec, Src0, One, sq, C0, C1, lower
    from concourse.dve_spec import _has_src1 as has_src1
    from concourse.dve_uop import DveOpSpec

    have = {op.name: op for op in dve_ops.OPS}
    if "ATTN_EXP_BASE" in have:
        return have["ATTN_EXP_BASE"], have["ATTN_EXP_SQ6"]

    u = Src0 * C0
    base = Spec(
        body=(One + u) + sq(u) * C1,
        reference=lambda in0, in1, c0, c1, c2:
            (1.0 + in0 * c0) + (in0 * c0) ** 2 * c1,
    )
    sq6 = Spec(
        body=sq(sq(sq(sq(sq(sq(Src0)))))),
        reference=lambda in0, in1, c0, c1, c2: in0 ** 64,
    )
    out = []
    for name, sp in (("ATTN_EXP_BASE", base), ("ATTN_EXP_SQ6", sq6)):
        shas = {}
        for ver in ("v3", "v4"):
            try:
                s = DveOpSpec(name=name, opcode=1, uops=lower(sp, ver=ver),
                              rd1_en=has_src1(sp))
                shas[ver] = s.sha(ver)
            except Exception:
                pass
        op = dve_ops.DveOp(name, sp, subdim=False, uops_sha=shas)
        dve_ops.OPS.append(op)
        dve_ops.CUSTOM_DVE_SPECS[name] = sp
        dve_ops._SUB_OPCODE_FOR_NAME[name] = (
            max(dve_ops._SUB_OPCODE_FOR_NAME.values()) + 1)
        out.append(op)
    return out[0], out[1]


def _deint_perm():
    """o' -> o source index: within each 64-block, evens first then odds."""
    d_order = list(range(0, DK, 2)) + list(range(1, DK, 2))  # position d' -> d
    perm = np.zeros(D, dtype=np.int64)
    for c in range(H):
        for dp, d in enumerate(d_order):
            perm[c * DK + dp] = c * DK + d
    return perm


def _sbuf_shuffle(wT):
    """[1024, 1024] -> [128, 8*1024] so each SBUF partition line is one
    contiguous 16KB DRAM run (128 fat DMA descriptors instead of 1024)."""
    return np.ascontiguousarray(
        wT.reshape(8, 128, wT.shape[1]).transpose(1, 0, 2).reshape(128, -1))


def _host_arrays(Wq, Wk, Wv, Wo):
    bf = ml_dtypes.bfloat16
    perm = _deint_perm()
    wqT = _sbuf_shuffle(np.ascontiguousarray(Wq[perm, :].T).astype(bf))
    wkT = _sbuf_shuffle(np.ascontiguousarray(Wk[perm, :].T).astype(bf))
    wvT = _sbuf_shuffle(np.ascontiguousarray(Wv.T).astype(bf))
    woT = _sbuf_shuffle(np.ascontiguousarray(Wo.T).astype(bf))

    # RoPE tables in the [Aev, Aod, Bev, Bod] partition grouping (32 rows each;
    # identical for both heads since the angle depends only on s).
    # Both Q and K are stored c-major (u = 256*c + r <-> s = 16*r + c), so one
    # u-ordered table pair serves both.
    j = np.arange(0, DK, 2, dtype=np.float64) / DK
    inv_freq = 1.0 / (ROPE_THETA ** j)                     # [32]
    u = np.arange(S)
    s_of_u = 16 * (u % 256) + u // 256
    angq = np.outer(inv_freq, s_of_u)                      # [32, S] u-ordered
    angk = np.outer(inv_freq, np.arange(S))                # [32, S] s-ordered
    csq1 = np.concatenate([np.cos(angq)] * 4, 0).astype(bf)
    csq2 = np.concatenate([-np.sin(angq), np.sin(angq)] * 2, 0).astype(bf)
    csk1 = np.concatenate([np.cos(angk)] * 4, 0).astype(bf)
    csk2 = np.concatenate([-np.sin(angk), np.sin(angk)] * 2, 0).astype(bf)
    # 0/1 triangle mask applied multiplicatively to exp'd P:
    # row p = local sk (plain), col j = 8*cq + rq (c-major sq)
    p = np.arange(128)
    cq, rq = np.arange(128) // 8, np.arange(128) % 8
    sq_loc = 16 * rq + cq                                  # [128]
    tri01 = np.where(p[:, None] <= sq_loc[None, :], 1.0, 0.0).astype(bf)
    # duplicated for both chunk slots so one gpsimd op masks the whole chunk
    tri01 = np.concatenate([tri01, tri01], axis=1)
    return wqT, wkT, wvT, woT, csq1, csq2, csk1, csk2, tri01


def _build_program(dbg=False):
    import concourse.bass as bass
    import concourse.tile as tile
    from concourse import bacc, mybir

    EXPB = EXPS = None
    if DVE_EXP_EVERY:
        EXPB, EXPS = _register_dve_exp()

    f32 = mybir.dt.float32
    bf16 = mybir.dt.bfloat16
    EXP = mybir.ActivationFunctionType.Exp
    CPY = mybir.ActivationFunctionType.Copy

    nc = bacc.Bacc("TRN2", target_bir_lowering=False, debug=False,
                   num_devices=NC_N)

    xT = nc.dram_tensor("xT", [128, 8 * RPC], bf16, kind="ExternalInput").ap()
    wq = nc.dram_tensor("wqT", [128, 8 * D], bf16, kind="ExternalInput").ap()
    wk = nc.dram_tensor("wkT", [128, 8 * D], bf16, kind="ExternalInput").ap()
    wv = nc.dram_tensor("wvT", [128, 8 * D], bf16, kind="ExternalInput").ap()
    wo = nc.dram_tensor("woT", [128, 8 * D], bf16, kind="ExternalInput").ap()
    cs1d = nc.dram_tensor("cs1", [128, S], bf16, kind="ExternalInput").ap()
    cs2d = nc.dram_tensor("cs2", [128, S], bf16, kind="ExternalInput").ap()
    cs3d = nc.dram_tensor("cs3", [128, S], bf16, kind="ExternalInput").ap()
    cs4d = nc.dram_tensor("cs4", [128, S], bf16, kind="ExternalInput").ap()
    trid = nc.dram_tensor("tri", [128, 256], bf16, kind="ExternalInput").ap()
    y = nc.dram_tensor("y", [RPC, D], f32, kind="ExternalOutput").ap()
    vfd = nc.dram_tensor("vflat_scratch", [RPC, D], bf16).ap()

    with tile.TileContext(nc) as tc:
        with (
            tc.tile_pool(name="big", bufs=3) as big,        # wq/wk/wv -> outTA/outTB/y_sb
            tc.tile_pool(name="wo", bufs=1) as wop,
            tc.tile_pool(name="xp", bufs=1) as xp,
            tc.tile_pool(name="qk", bufs=2) as qkp,          # qraw, kraw (become rot in place)
            tc.tile_pool(name="cs", bufs=4) as csp,          # rope tables
            tc.tile_pool(name="sw", bufs=2) as swp,          # K-rope swap scratch
            tc.tile_pool(name="swq", bufs=1) as swqp,        # Q-rope swap scratch
            tc.tile_pool(name="vf", bufs=1) as vfp,
            tc.tile_pool(name="vsb", bufs=2) as vsbp,
            tc.tile_pool(name="stg", bufs=1) as stgp,        # K r-tail staging
            tc.tile_pool(name="mask", bufs=1) as maskp,
            tc.tile_pool(name="pp", bufs=18) as ppool,       # exp'd P chunks
            tc.tile_pool(name="ct", bufs=4) as ctp,          # concatT per (h, rt)
            tc.tile_pool(name="norm", bufs=2) as normp,      # dn row + bcast block
        ):
            # ---------- phase 0: loads ----------
            # DRAM pre-shuffled on host so every 512-col chunk is a 1KB
            # contiguous run per partition line.  All loads go on the sync
            # ring (scalar-ring dma_starts backpressure the ScalarE FIFO
            # and stall the PSUM-evac copies behind them).  Emission order
            # is first-use order for the K-first phase schedule:
            # x, wk, wq, cs34, wv, cs12, tri (wo rides behind the gather).
            def loadc(dst, src, n_chunks):
                w = dst.shape[-1] // n_chunks
                for ch in range(n_chunks):
                    nc.sync.dma_start(dst[:, ch * w:(ch + 1) * w],
                                      src[:, ch * w:(ch + 1) * w])

            xsb = xp.tile([128, 8 * RPC], bf16, tag="x")           # [p, kt*512+r]
            wq_sb = big.tile([128, 8 * D], bf16, tag="big", name="w_wq")
            wk_sb = big.tile([128, 8 * D], bf16, tag="big", name="w_wk")
            wv_sb = big.tile([128, 8 * D], bf16, tag="big", name="w_wv")
            wo_sb = wop.tile([128, 8 * D], bf16, tag="wo")
            # interleave x and wk chunks so K proj's kt=0 inputs land first
            for kt in range(8):
                nc.sync.dma_start(xsb[:, kt * RPC:(kt + 1) * RPC],
                                  xT[:, kt * RPC:(kt + 1) * RPC])
                nc.sync.dma_start(wk_sb[:, kt * D: kt * D + 512],
                                  wk[:, kt * D: kt * D + 512])
                nc.sync.dma_start(wk_sb[:, kt * D + 512:(kt + 1) * D],
                                  wk[:, kt * D + 512:(kt + 1) * D])
            loadc(wq_sb[:], wq, 16)
            cs3_sb = csp.tile([128, S], bf16, tag="cs")
            loadc(cs3_sb[:], cs3d, 8)
            cs4_sb = csp.tile([128, S], bf16, tag="cs")
            loadc(cs4_sb[:], cs4d, 8)
            loadc(wv_sb[:], wv, 16)
            cs1_sb = csp.tile([128, S], bf16, tag="cs")
            loadc(cs1_sb[:], cs1d, 8)
            cs2_sb = csp.tile([128, S], bf16, tag="cs")
            loadc(cs2_sb[:], cs2d, 8)
            tri_sb = maskp.tile([128, 256], bf16, tag="mask")
            nc.sync.dma_start(tri_sb[:], trid[:])

            qraw = qkp.tile([128, S], bf16, tag="qk")   # [Aev,Aod,Bev,Bod] x s
            kraw = qkp.tile([128, S], bf16, tag="qk")

            # vsb tiles early (gpsimd memset of the ones column is off-path)
            vsbs = []
            for h in range(HPC):
                vsb = vsbp.tile([128, 65 * NT_SK], bf16, tag="vsb")
                nc.gpsimd.memset(vsb[:], 1.0)   # ones col at 65T+64 survives
                vsbs.append(vsb)

            from contextlib import ExitStack
            _ps_stack = ExitStack()
            with _ps_stack:
                # projections get all 8 PSUM banks; the scores pool only
                # opens once the projection pool closes (pre-attention)
                with tc.tile_pool(name="psproj", bufs=8, space="PSUM") as psp:
                    # ---------- phase 1b: K projection FIRST (s-ordered
                    # storage; strided copies split across both engines).
                    # K leads so rope chunk 0 + the first QK scores come as
                    # early as possible; Q/V projections then overlap the
                    # rope chain on DVE. ----------
                    kv_raw = kraw[:].rearrange("p (r c) -> p r c", c=16)
                    stgk = stgp.tile([128, 1024], bf16, tag="stg")

                    def kcopy2(pq, ot):
                        # pass 2: r in [64, 192) — emitted 4 ots behind so
                        # rope chunk 0 (which only needs r < 64) can jump the
                        # copy queues
                        c0 = 2 * ot
                        nc.scalar.activation(kv_raw[0:64, 64:192, c0],
                                             pq[0:64, 64:192], CPY)
                        nc.vector.tensor_copy(kv_raw[64:128, 64:192, c0],
                                              pq[0:64, 320:448])
                        nc.vector.tensor_copy(kv_raw[0:64, 64:192, c0 + 1],
                                              pq[64:128, 64:192])
                        nc.scalar.activation(kv_raw[64:128, 64:192, c0 + 1],
                                             pq[64:128, 320:448], CPY)

                    kpqs = []
                    for ot in range(8):
                        if ot >= 4:
                            kcopy2(kpqs[ot - 4], ot - 4)
                        pq = psp.tile([128, RPC], f32, tag="ps")
                        for kt in range(8):
                            nc.tensor.matmul(
                                pq[:],
                                wk_sb[:, kt * D + ot * 128: kt * D + ot * 128 + 128],
                                xsb[:, kt * RPC: (kt + 1) * RPC],
                                start=(kt == 0), stop=(kt == 7),
                            )
                        c0 = 2 * ot
                        # pass 1: r < 64 (s < 1024: rope chunk 0's input),
                        # split across both engines
                        nc.scalar.activation(kv_raw[0:64, 0:64, c0],
                                             pq[0:64, 0:64], CPY)
                        nc.vector.tensor_copy(kv_raw[64:128, 0:64, c0],
                                              pq[0:64, 256:320])
                        nc.vector.tensor_copy(kv_raw[0:64, 0:64, c0 + 1],
                                              pq[64:128, 0:64])
                        nc.scalar.activation(kv_raw[64:128, 0:64, c0 + 1],
                                             pq[64:128, 256:320], CPY)
                        # r-tail [192, 256) staged contiguously, scattered
                        # later under the attention stream
                        stgs = stgk[:, 128 * ot: 128 * ot + 128].rearrange(
                            "p (g r) -> p g r", g=2)
                        pqs = pq[:].rearrange("p (g r) -> p g r", g=2)
                        nc.scalar.activation(stgs[:], pqs[:, :, 192:256], CPY)
                        kpqs.append(pq)
                    for ot in range(4, 8):
                        kcopy2(kpqs[ot], ot)
                    # K rope chunks 0-2 (s < 3072, covers q-blocks 0-5);
                    # chunk 3 is deferred with the r-tail scatter
                    kv_stg = stgk[:].rearrange("p (ot g r) -> p ot g r",
                                               ot=8, g=2)

                    def emit_krope(kc):
                        lo, hi = 1024 * kc, 1024 * (kc + 1)
                        swk = swp.tile([128, 1024], bf16, tag="sw",
                                       name=f"swk{kc}")
                        nc.vector.tensor_copy(swk[0:32, :], kraw[32:64, lo:hi])
                        nc.vector.tensor_copy(swk[32:64, :], kraw[0:32, lo:hi])
                        nc.vector.tensor_copy(swk[64:96, :], kraw[96:128, lo:hi])
                        nc.vector.tensor_copy(swk[96:128, :], kraw[64:96, lo:hi])
                        nc.vector.tensor_mul(swk[:], swk[:], cs4_sb[:, lo:hi])
                        nc.vector.tensor_mul(kraw[:, lo:hi], kraw[:, lo:hi],
                                             cs3_sb[:, lo:hi])
                        nc.vector.tensor_add(kraw[:, lo:hi], kraw[:, lo:hi],
                                             swk[:])

                    emit_krope(0)

                    # ---------- phase 1a: Q projection (c-major storage,
                    # contiguous copies, split 2-ScalarE / 2-DVE) ----------
                    for ot in range(8):
                        pq = psp.tile([128, RPC], f32, tag="ps")
                        for kt in range(8):
                            nc.tensor.matmul(
                                pq[:],
                                wq_sb[:, kt * D + ot * 128: kt * D + ot * 128 + 128],
                                xsb[:, kt * RPC: (kt + 1) * RPC],
                                start=(kt == 0), stop=(kt == 7),
                            )
                        c0 = 2 * ot
                        u0, u1 = 256 * c0, 256 * (c0 + 1)
                        nc.scalar.activation(qraw[0:64, u0:u0 + 256],
                                             pq[0:64, 0:256], CPY)
                        nc.vector.tensor_copy(qraw[64:128, u0:u0 + 256],
                                              pq[0:64, 256:512])
                        nc.vector.tensor_copy(qraw[0:64, u1:u1 + 256],
                                              pq[64:128, 0:256])
                        nc.scalar.activation(qraw[64:128, u1:u1 + 256],
                                             pq[64:128, 256:512], CPY)
                    # Q rope, full width: runs on DVE while the V projection
                    # occupies PE
                    swq = swqp.tile([128, S], bf16, tag="swq", name="swq")
                    nc.vector.tensor_copy(swq[0:32, :], qraw[32:64, :])
                    nc.vector.tensor_copy(swq[32:64, :], qraw[0:32, :])
                    nc.vector.tensor_copy(swq[64:96, :], qraw[96:128, :])
                    nc.vector.tensor_copy(swq[96:128, :], qraw[64:96, :])
                    nc.vector.tensor_mul(swq[:], swq[:], cs2_sb[:])
                    nc.vector.tensor_mul(qraw[:], qraw[:], cs1_sb[:])
                    nc.vector.tensor_add(qraw[:], qraw[:], swq[:])

                    # (V projection moved into the attention prelude below,
                    # interleaved with the first chunks' QK+exp)

                    def scat(ot):
                        c0 = 2 * ot
                        nc.vector.tensor_copy(kv_raw[0:64, 192:256, c0],
                                              kv_stg[0:64, ot, 0, :])
                        nc.vector.tensor_copy(kv_raw[64:128, 192:256, c0],
                                              kv_stg[0:64, ot, 1, :])
                        nc.vector.tensor_copy(kv_raw[0:64, 192:256, c0 + 1],
                                              kv_stg[64:128, ot, 0, :])
                        nc.vector.tensor_copy(kv_raw[64:128, 192:256, c0 + 1],
                                              kv_stg[64:128, ot, 1, :])

                    qrot, krot = qraw, kraw

                # ---------- phase 4: attention ----------
                pssc = _ps_stack.enter_context(
                    tc.tile_pool(name="pssc", bufs=2, space="PSUM"))
                qv = [qrot[64 * h: 64 * h + 64, :].rearrange(
                    "p (c r) -> p c r", c=16) for h in range(HPC)]
                triv = tri_sb[:].rearrange("p (j c r) -> p j c r",
                                           j=2, r=8)

                all_chunks = []
                for b in range(8):
                    nt = 4 * (b + 1)
                    slots = [(t, h) for t in range(nt) for h in range(HPC)]
                    for i in range(0, len(slots), 2):
                        all_chunks.append((b, slots[i:i + 2],
                                           i + 2 >= len(slots)))

                # deadline-ordered deferred DVE work: K-rope chunks ahead of
                # the sk tiles that need them, K r-tail scatter in between
                dve_bg = ([lambda: emit_krope(1), lambda: emit_krope(2)]
                          + [(lambda ot=ot: scat(ot)) for ot in range(8)]
                          + [lambda: emit_krope(3)])

                def emit_chunk(ci, b, chunk):
                    ps = pssc.tile([128, 1024], f32, tag="sc")
                    pch = ppool.tile([128, 1024], bf16, tag="pp")
                    # slot layout: uniform c-major-32 groups; valid r-range
                    # [rmin, 32) per c-group, garbage at [0, rmin)
                    for j, (t, h) in enumerate(chunk):
                        rmin = 8 * max(0, t - 4 * b)
                        psv = ps[:, 512 * j: 512 * (j + 1)].rearrange(
                            "p (c r) -> p c r", r=32)
                        nc.tensor.matmul(
                            psv[:, :, rmin:32],
                            krot[64 * h: 64 * h + 64,
                                 128 * t: 128 * t + 128],
                            qv[h][:, :, 32 * b + rmin: 32 * (b + 1)],
                            start=True, stop=True,
                        )
                    rmin0 = 8 * max(0, chunk[0][0] - 4 * b)
                    if rmin0:
                        # diagonal chunks: skip exp'ing the [0, rmin)
                        # garbage cols (never read downstream)
                        psv4 = ps[:].rearrange("p (j c r) -> p j c r",
                                               j=2, r=32)
                        pcv4 = pch[:].rearrange("p (j c r) -> p j c r",
                                                j=2, r=32)
                        nc.scalar.activation(pcv4[:, :, :, rmin0:32],
                                             psv4[:, :, :, rmin0:32],
                                             EXP, scale=0.125)
                    else:
                        nc.scalar.activation(pch[:], ps[:], EXP,
                                             scale=0.125)
                    if chunk[0][0] >= 4 * b:
                        # both slots share t -> one fused mask multiply
                        pm = pch[:].rearrange(
                            "p (j c r) -> p j c r", j=2,
                            r=32)[:, :, :, rmin0:rmin0 + 8]
                        nc.gpsimd.tensor_mul(pm, pm, triv)
                    if dve_bg and ci % 2 == 0:
                        dve_bg.pop(0)()
                    return pch

                # ---------- phase 1v (interleaved): V projection + reshape
                # woven between the first N_EARLY chunks' QK+exp so the
                # ScalarE exp stream starts ~25us earlier; those chunks'
                # PV waits in pvq (pch pool keeps them live) until the vsb
                # tiles land.  rt order (0,2,1,3): both heads' first halves
                # reach vsb first.  Reshape goes through DRAM (gather).
                N_EARLY = 12
                pvq = []
                ecur = [0]

                def emit_early(n):
                    while n > 0 and ecur[0] < N_EARLY:
                        ci = ecur[0]
                        eb, echunk, elast = all_chunks[ci]
                        pch = emit_chunk(ci, eb, echunk)
                        pvq.append((eb, echunk, pch, elast))
                        ecur[0] += 1
                        n -= 1

                vflat = vfp.tile([128, 4 * D], bf16, tag="vf")
                vfd_v = vfd.rearrange("(rt p) o -> p rt o", p=128)
                vld = vfd.rearrange("(h T a) (c d) -> h a c T d", h=2,
                                    T=NT_SK, a=8, c=16, d=DK)
                _vps = ExitStack()
                vpsp = _vps.enter_context(
                    tc.tile_pool(name="psv", bufs=2, space="PSUM"))
                for rt in (0, 2, 1, 3):
                    for ob in range(2):
                        pv = vpsp.tile([128, 512], f32, tag="psv")
                        for kt in range(8):
                            nc.tensor.matmul(
                                pv[:],
                                xsb[:, kt * RPC + rt * 128: kt * RPC + rt * 128 + 128],
                                wv_sb[:, kt * D + ob * 512: kt * D + ob * 512 + 512],
                                start=(kt == 0), stop=(kt == 7),
                            )
                        # PSUM evac on DVE: ScalarE now runs the exp stream
                        nc.vector.tensor_copy(
                            vflat[:, rt * D + ob * 512: rt * D + ob * 512 + 512],
                            pv[:])
                        emit_early(1)
                    nc.sync.dma_start(vfd_v[:, rt, :],
                                      vflat[:].rearrange(
                                          "p (rt o) -> p rt o", rt=4)[:, rt, :])
                    # quirky map: vfd rows [256h+128*half, +128) = head h,
                    # tiles T in [16*half, 16*half+16)
                    h, half = divmod(rt, 2)
                    T0 = 16 * half
                    dstv = vsbs[h][:].rearrange("(a c) (T d) -> a c T d",
                                                a=8, c=16, T=NT_SK, d=65)
                    for a in range(8):
                        nc.sync.dma_start(dstv[a, :, T0:T0 + 16, 0:DK],
                                          vld[h, a, :, T0:T0 + 16, :])
                    emit_early(1)
                emit_early(N_EARLY)
                _vps.close()
                # wo rides behind the V gather: first needed at the b=4
                # oproj drip, well after the gather drains
                loadc(wo_sb[:], wo, 16)

                with tc.tile_pool(name="psout", bufs=4, space="PSUM") as psout:
                    y_sb = big.tile([128, 4 * D], f32, tag="big")
                    yv = y.rearrange("(g p) o -> p g o", p=128)

                    # normalize writes land directly in concatT (ct) layout:
                    # ct[(c%2)*64+d, 128*(c//2) + r%128] per (h, rt=r//128),
                    # so the o-projection needs no gather copies at all
                    cts = {}
                    for h in range(HPC):
                        for rt in range(2):
                            cts[(h, rt)] = ctp.tile([128, 8 * 128], bf16,
                                                    tag="ct",
                                                    name=f"ct{h}_{rt}")

                    outps = {}
                    opvs = {}

                    def ensure_outp(b):
                        if b in outps:
                            return
                        outps[b] = [psout.tile([65, 512], f32, tag="out",
                                               name=f"outp{b}_{hh}")
                                    for hh in range(HPC)]
                        opvs[b] = [outps[b][hh][:].rearrange(
                            "p (c r) -> p c r", r=32) for hh in range(HPC)]

                    def emit_pv(b, chunk, pch):
                        nt = 4 * (b + 1)
                        for j, (t, h) in enumerate(chunk):
                            rmin = 8 * max(0, t - 4 * b)
                            pcv = pch[:, 512 * j: 512 * (j + 1)].rearrange(
                                "p (c r) -> p c r", r=32)
                            nc.tensor.matmul(
                                opvs[b][h][:, :, rmin:32],
                                vsbs[h][:, 65 * t: 65 * t + 65],
                                pcv[:, :, rmin:32],
                                start=(t == 0), stop=(t == nt - 1),
                            )

                    def emit_norm(b, h):
                        # row 64 of outp holds the softmax denominators (ones
                        # column of the [V|1] stationary); broadcast 1/d on
                        # the idle GpSimd and scale straight into ct layout
                        rt, ro = b // 4, 32 * (b % 4)
                        outp = outps[b][h]
                        nrm = normp.tile([128, 1024], f32, tag="norm",
                                         name=f"nrm{b}_{h}")
                        nc.vector.tensor_copy(nrm[0:1, 512:1024],
                                              outp[64:65, :])
                        nc.vector.reciprocal_approx_fast(
                            out=nrm[0:1, 0:512], in_=nrm[0:1, 512:1024])
                        nc.vector.stream_shuffle(nrm[64:96, 0:512],
                                                 nrm[0:32, 0:512], [0] * 32)
                        nc.vector.stream_shuffle(nrm[96:128, 0:512],
                                                 nrm[0:32, 0:512], [0] * 32)
                        # outp col = 32c + r with c = 2*tp + c2; ct col =
                        # 128*tp + 32*(b%4) + r, partition half by c2
                        ctv = cts[(h, rt)][:].rearrange(
                            "p (tp j) -> p tp j", j=128)
                        opv2 = outp[:].rearrange("p (tp c2 r) -> p tp c2 r",
                                                 c2=2, r=32)
                        bcv = nrm[64:128, 0:512].rearrange(
                            "p (tp c2 r) -> p tp c2 r", c2=2, r=32)
                        for c2 in range(2):
                            nc.vector.tensor_mul(
                                ctv[64 * c2: 64 * c2 + 64, :, ro:ro + 32],
                                opv2[0:64, :, c2], bcv[:, :, c2])

                    oproj_bg = []   # deferred o-proj work, drip-fed to PE

                    def make_oproj(h, rt):
                        g = 2 * h + rt
                        ct = cts[(h, rt)]
                        work = []
                        cell = {}   # py allocated lazily at first matmul so a
                        # PSUM slot is only held while the chain is in flight
                        for ob in range(2):
                            for tp in range(8):
                                def mm(tp=tp, ob=ob):
                                    if tp == 0:
                                        cell[ob] = psout.tile(
                                            [128, 512], f32, tag="out",
                                            name=f"py{g}_{ob}")
                                    nc.tensor.matmul(
                                        cell[ob][:],
                                        ct[:, 128 * tp: 128 * tp + 128],
                                        wo_sb[:, tp * D + ob * 512:
                                              tp * D + ob * 512 + 512],
                                        start=(tp == 0), stop=(tp == 7),
                                    )
                                work.append(mm)

                            def fin(ob=ob):
                                nc.vector.tensor_copy(
                                    y_sb[:, g * D + ob * 512:
                                         g * D + ob * 512 + 512],
                                    cell[ob][:])
                                if ob == 1:
                                    nc.sync.dma_start(
                                        yv[:, g, :], y_sb[:, g * D:(g + 1) * D])
                            work.append(fin)
                        return work

                    def flush_pv(n):
                        # drain the oldest queued chunks' PV (in order);
                        # norm fires on each block's last chunk
                        while pvq and n > 0:
                            pb, pchunk, ppch, plast = pvq.pop(0)
                            ensure_outp(pb)
                            emit_pv(pb, pchunk, ppch)
                            if plast and pb != 7:
                                for hh in range(HPC):
                                    emit_norm(pb, hh)
                                if pb == 3:
                                    oproj_bg.extend(make_oproj(0, 0))
                                    oproj_bg.extend(make_oproj(1, 0))
                            n -= 1

                    for ci in range(N_EARLY, len(all_chunks)):
                        b, chunk, is_last = all_chunks[ci]
                        pch = emit_chunk(ci, b, chunk)
                        pvq.append((b, chunk, pch, is_last))
                        # steady state keeps lag-1; the early backlog drains
                        # at 2 per chunk
                        flush_pv(2 if len(pvq) > 6 else
                                 (1 if len(pvq) > 1 else 0))
                        if oproj_bg and b >= 4 and ci % 2 == 0:
                            oproj_bg.pop(0)()
                    flush_pv(len(pvq))
                    while oproj_bg:
                        oproj_bg.pop(0)()
                    # tail: both norms first, then interleave the two final
                    # oproj groups so each group's LDW/sem waits hide under
                    # the other group's matmuls
                    for hh in range(HPC):
                        emit_norm(7, hh)
                    w0 = make_oproj(0, 1)
                    w1 = make_oproj(1, 1)
                    for wa, wb in zip(w0, w1):
                        wa()
                        wb()

    nc.compile()
    return nc


def kernel(**inputs):
    x = np.asarray(inputs["x"], dtype=np.float32)     # [1, 4096, 1024]
    Wq = np.asarray(inputs["Wq"], dtype=np.float32)
    Wk = np.asarray(inputs["Wk"], dtype=np.float32)
    Wv = np.asarray(inputs["Wv"], dtype=np.float32)
    Wo = np.asarray(inputs["Wo"], dtype=np.float32)
    # biases are structurally zero in this problem; fold anyway if nonzero
    for bn in ("bq", "bk", "bv", "bo"):
        bv_ = np.asarray(inputs.get(bn, 0.0))
        assert np.all(bv_ == 0.0), f"{bn} nonzero: unsupported"

    from concourse.bass_utils import run_bass_kernel_spmd

    if "nc" not in _CACHE:
        _CACHE["nc"] = _build_program()
    nc = _CACHE["nc"]

    bf = ml_dtypes.bfloat16
    wqT, wkT, wvT, woT, csq1, csq2, csk1, csk2, tri01 = _host_arrays(
        Wq, Wk, Wv, Wo)
    shared = {"wqT": wqT, "wkT": wkT, "wvT": wvT, "woT": woT,
              "cs1": csq1, "cs2": csq2, "cs3": csk1, "cs4": csk2,
              "tri": tri01}
    xf = x.reshape(S, D)
    in_maps = []
    for i in range(NC_N):
        xTi = _sbuf_shuffle(
            np.ascontiguousarray(xf[i * RPC:(i + 1) * RPC, :].T).astype(bf))
        in_maps.append(dict(shared, xT=xTi))

    trace = bool(int(os.environ.get("BASS_KERNEL_TRACE", "0")))
    res = run_bass_kernel_spmd(nc, in_maps, core_ids=list(range(NC_N)),
                               trace=trace)
    _CACHE["last_res"] = res
    if trace and res.exec_time_ns is not None:
        print(f"HW exec time: {res.exec_time_ns} ns")
        _CACHE["exec_time_ns"] = res.exec_time_ns
        _CACHE["trace"] = res.instructions_and_trace
    out = np.concatenate([res.results[i]["y"] for i in range(NC_N)], axis=0)
    return out.reshape(1, S, D).astype(np.float32)

